# revision 33
# baseline (speedup 1.0000x reference)
"""ChebyKAN linear layer on 8 Trainium2 NeuronCores.

Computation: out[b,o] = sum_{i,d} T_d(tanh(x[b,i])) * coef[i,o,d]
  == sum_d T_d(tanh(x)) @ C_d   (9 accumulated 8192x1024x1024 matmuls)

Strategy:
  - Data-parallel over batch: core c handles rows [c*1024, (c+1)*1024).
  - Host pre-transposes each core's x slice to (in_features, batch) layout so
    the contraction dim (i) lands on SBUF partitions, and repacks the
    coefficients to (d, i, o): bf16 for degrees 1..5, fp8 e4m3 (k-tile
    pairs on a separate axis for DoubleRow) for degrees 6..8. ALL weights
    are pre-scaled by 2^12 — exact for bf16 (power-of-2 scale leaves the
    mantissa untouched) and required for fp8 (coef sigma ~1.1e-4 would land
    in e4m3's subnormal/flush range unscaled) — so every matmul accumulates
    into one PSUM group at a single scale.
  - On-chip: ACT computes tanh in fp32, DVE runs the Chebyshev recursion
    T_d = 2 t T_{d-1} - T_{d-2} in fp32 (scalar_tensor_tensor fuses the
    2*t*T_{d-1} product into one op), ACT casts T_1..T_5 to bf16 and
    T_6..T_8 into slot i%2 of fp8 e4m3 DoubleRow pair tiles [128, 2, 512].
  - PE: per half, one PSUM accumulation group per (b-chunk, o-half) bank:
    bf16 matmuls for degrees 1..5 stream per k-tile; after each odd k-tile
    the pair's fp8 DoubleRow matmuls (2 k-tiles per instruction, 2x MACs
    per cycle-row) are interleaved into the same group. One DVE drain per
    bank computes psum * 2^-12 + bias (d=0 term, host-precomputed fp32,
    partition-replicated) and DMAs out.
  - Per core the 1024-row batch is processed in two 512-column halves; each
    half keeps its full output (4 b-chunks x 2 o-halves) resident in all
    8 PSUM banks while the k-blocks accumulate into it.

Numerics (validated on HW, matches a numpy simulation of the exact
quantization to ~1e-6): rel l2 error vs fp32 reference 1.80e-2 (threshold
2e-2). fp8 on 3 of 8 degrees contributes ~sqrt(3)*1.04e-2, the bf16
remainder ~1.8e-3; 4 fp8 degrees would be 2.07e-2 — over threshold.

Performance notes (8-core SPMD, measured via on-device For_i loop slope
because the axon tunnel's ~80-95 ms RPC overhead hides the kernel and NTFF
profiling is unavailable through it; cross-process slope noise is +-5 us):
  - Measured matmult throughput on this part (mmonly/mmrows256 diagnostic
    variants, pure PE stream, no deps) is 0.51 ns/row — 1.96 GHz effective,
    NOT the 2.4 GHz nominal — and is linear in rows with ~zero per-
    instruction overhead. The all-bf16 kernel (1024 512-row matmuls,
    267 us of stream) measured 269-283 us: already ~97% of the PE roofline.
  - Ldweights are nearly free when sync-free (hidden in the matmul
    pipeline): deduping 187 extra reloads moved the time < 2 us. The
    engine-aware dedup pass below keeps them at the 1-per-(bc-chunk) floor.
  - DMA (weights re-streamed per half, 33.6 MB/iter/core), PSUM drains and
    half boundaries are all second-order (nodma/nodrain variants within
    noise of base).
  - fp8 DoubleRow halves the row count of each converted degree. Measured:
    all-bf16 ~274 us; degrees {7,8} fp8 (two-wave drain) 248.8 us; degrees
    {6,7,8} fp8 (two-wave) 234.4 us; single-group (current) removes the
    wave-split drains and bubbles.
  - A 1024-free matmult (2 PSUM banks) is ISA-illegal (s3d3_mm_num_elements
    codegen assert); pure-fp8 everything fails accuracy (3.1e-2).
"""

import numpy as np
import ml_dtypes

BATCH = 8192
IN_F = 1024
OUT_F = 1024
DEG = 8  # degree; DEG+1 coefficients per (i,o)
FP8_DEGS = (6, 7, 8)  # degrees computed as fp8 e4m3 DoubleRow
N_CORES = 8
B_CORE = BATCH // N_CORES  # 1024
P = 128
HALF = 512  # batch columns processed per PSUM-resident output block
NI = IN_F // P  # 8 contraction tiles
NBC = HALF // P  # 4 b-chunks per half
NOH = OUT_F // 512  # 2 output halves of 512
N_HALF = B_CORE // HALF  # 2

_CACHED_NC = {}


def _build_bass(loop_r=None, variant=""):
    """Build the Bass program. loop_r wraps the whole compute in a hardware
    For loop of loop_r iterations (benchmark-only; slope over loop_r gives
    per-iteration HW time since the axon RPC overhead is per-call)."""
    import contextlib

    import concourse.mybir as mybir
    import concourse.tile as tile
    from concourse import bacc

    f32 = mybir.dt.float32
    bf16 = mybir.dt.bfloat16
    mult = mybir.AluOpType.mult
    sub = mybir.AluOpType.subtract
    Tanh = mybir.ActivationFunctionType.Tanh

    import json as _json

    def _dedup_ldweights(b):
        """Remove InstLdweights that reload the identical stationary operand
        still held by the PE array. Tile emits one Ldweights per matmul, so a
        weight reused by consecutive matmuls is loaded twice; each redundant
        load costs ~55 ns of serial PE time. Only sync-free exact duplicates
        are removed. Instructions on OTHER engine queues interleaved in the
        block's program order cannot disturb the PE weight registers, so they
        do not reset the tracked key; the only PE-queue instructions between
        a duplicate pair are Matmult (does not disturb weights) and sync-free
        EventSemaphore. An EventSemaphore with a wait is treated as a fence
        (conservative: it could order an SBUF rewrite of the stationary)."""
        n_removed = 0
        PE = mybir.EngineType.PE
        for fn in b.m.functions:
            for blk in fn.blocks:
                last_key = None
                keep = []
                for inst in blk.instructions:
                    if isinstance(inst, mybir.InstLdweights):
                        d = _json.loads(
                            mybir.instruction_to_pretty_json_string(inst)
                        )
                        si = d.get("sync_info") or {}
                        has_sync = bool(
                            si.get("on_wait") or si.get("on_update")
                        )
                        key = _json.dumps(
                            [
                                d.get("ins"),
                                d.get("perf_mode"),
                                d.get("is_transpose"),
                                d.get("tile_position"),
                                d.get("tile_size"),
                            ],
                            sort_keys=True,
                        )
                        if key == last_key and not has_sync:
                            n_removed += 1
                            continue
                        last_key = key
                    elif inst.engine != PE:
                        pass  # other-engine work; PE array unaffected
                    elif isinstance(inst, mybir.InstMatmult):
                        pass  # matmult does not disturb loaded weights
                    elif isinstance(inst, mybir.InstEventSemaphore):
                        si = inst.sync_info
                        if si is not None and si.on_wait:
                            # conservative fence: a PE-queue wait could
                            # order an SBUF rewrite of the stationary
                            last_key = None
                    else:
                        last_key = None
                    keep.append(inst)
                blk.instructions[:] = keep
        b._ldw_removed = n_removed

    class _Bacc(bacc.Bacc):
        def compile(self):
            super().compile()
            _dedup_ldweights(self)

    f8 = mybir.dt.float8e4

    nc = _Bacc(name="chebykan")
    xt = nc.dram_tensor("xt", (IN_F, B_CORE), f32, kind="ExternalInput")
    w = nc.dram_tensor("w", (DEG, IN_F, OUT_F), bf16, kind="ExternalInput")
    # degrees 7..8 as fp8 e4m3, k-tile pairs interleaved for DoubleRow:
    # w8[dd, j, k, s, o] = coef[(2j+s)*128 + k, o, 7+dd] * 2^12
    w8 = nc.dram_tensor(
        "w8", (len(FP8_DEGS), NI // 2, P, 2, OUT_F), f8, kind="ExternalInput"
    )
    brep = nc.dram_tensor("brep", (P, OUT_F), f32, kind="ExternalInput")
    out = nc.dram_tensor("out", (B_CORE, OUT_F), f32, kind="ExternalOutput")

    with (
        tile.TileContext(nc) as tc,
        tc.tile_pool(name="wpool", bufs=12) as wpool,
        tc.tile_pool(name="w8pool", bufs=6) as w8pool,
        tc.tile_pool(name="xpool", bufs=8) as xpool,
        tc.tile_pool(name="tanh", bufs=3) as tanpool,
        tc.tile_pool(name="rec", bufs=6) as rpool,
        tc.tile_pool(name="ch", bufs=16) as chpool,
        tc.tile_pool(name="ch8", bufs=16) as ch8pool,
        tc.tile_pool(name="const", bufs=1) as cpool,
        tc.tile_pool(name="outp", bufs=16) as opool,
        tc.tile_pool(name="psum", bufs=1, space="PSUM") as pspool,
    ):
        ones = cpool.tile([P, P], bf16)
        nc.vector.memset(ones[:], 1.0)
        # d=0 bias, pre-replicated across partitions on the host; added
        # during the PSUM drain instead of spending PE matmuls on it
        biasrep = cpool.tile([P, OUT_F], f32)
        nc.sync.dma_start(biasrep[:], brep[:, :])

        loop_cm = (
            tc.For_i(
                0,
                loop_r,
                1,
                hint_engines=(mybir.EngineType.PE, mybir.EngineType.SP),
            )
            if loop_r is not None
            else contextlib.nullcontext()
        )
        with loop_cm:
            _emit_body(nc, tc, xt, w, w8, out, ones, biasrep,
                       wpool, w8pool, xpool, tanpool, rpool, chpool, ch8pool,
                       opool, pspool, f32, bf16, f8, mult, sub, Tanh, variant)
    nc.finalize()
    return nc


def _emit_body(nc, tc, xt, w, w8, out, ones, biasrep,
               wpool, w8pool, xpool, tanpool, rpool, chpool, ch8pool,
               opool, pspool, f32, bf16, f8, mult, sub, Tanh, variant=""):
    import concourse.mybir as mybir
    add = mybir.AluOpType.add
    wide = variant == "wide"
    n_oh = 1 if variant == "halfmm" else NOH
    if variant in ("mmonly", "mmrows256"):
        # diagnostic: pure PE stream — same matmult/psum-group structure as
        # the real kernel but constant operands, no DMA/recursion/drains.
        # Measures sustained matmult throughput on this hardware.
        nfree = 256 if variant == "mmrows256" else 512
        wt0 = wpool.tile([P, OUT_F], bf16, tag="w")
        nc.sync.dma_start(wt0[:], w[0, 0:P, :])
        for h in range(N_HALF):
            ps = [
                [
                    pspool.tile([P, nfree], f32, tag=f"psm_{bc}_{oh}",
                                name=f"psm_{bc}_{oh}")
                    for oh in range(NOH)
                ]
                for bc in range(NBC)
            ]
            for i in range(NI):
                for d in range(1, DEG + 1):
                    start = i == 0 and d == 1
                    stop = i == NI - 1 and d == DEG
                    for bc in range(NBC):
                        for oh in range(NOH):
                            nc.tensor.matmul(
                                ps[bc][oh],
                                ones,
                                wt0[:, oh * nfree : (oh + 1) * nfree],
                                start=start,
                                stop=stop,
                            )
        return
    for h in range(N_HALF):
            if wide:
                # one [P, 1024] tile spans 2 PSUM banks; a single matmult
                # accumulates the full 1024-wide output row block
                ps = [
                    pspool.tile([P, OUT_F], f32, tag=f"psw_{bc}",
                                name=f"psw_{bc}")
                    for bc in range(NBC)
                ]
            else:
                ps = [
                    [
                        pspool.tile(
                            [P, 512], f32, tag=f"ps_{bc}_{oh}",
                            name=f"ps_{bc}_{oh}"
                        )
                        for oh in range(n_oh)
                    ]
                    for bc in range(NBC)
                ]
            bm = variant == "bm"
            use_fp8 = variant in ("", "fp8")
            n_bf = DEG - len(FP8_DEGS) if use_fp8 else DEG  # degrees in bf16
            ch8_list = []  # per k-tile pair j: {7: tile, 8: tile}
            for i in range(NI):
                last_i = i == NI - 1
                chds = {}
                wts = {}
                if use_fp8 and i % 2 == 0:
                    pair = {}
                    for dd in FP8_DEGS:
                        pair[dd] = ch8pool.tile(
                            [P, 2, HALF], f8, tag=f"ch8_{dd}",
                            name=f"ch8_{h}_{i}_{dd}",
                        )
                    ch8_list.append(pair)
                xti = xpool.tile([P, HALF], f32, tag="x")
                nc.sync.dma_start(
                    xti[:], xt[i * P : (i + 1) * P, h * HALF : (h + 1) * HALF]
                )
                t = tanpool.tile([P, HALF], f32, tag="t")
                nc.scalar.activation(t[:], xti[:], Tanh)

                tm2 = None  # T_{d-2} (fp32); None encodes T_0 == 1
                tm1 = t  # T_{d-1} (fp32)
                ch1 = None
                for d in range(1, DEG + 1):
                    last = d == DEG
                    in_fp8 = use_fp8 and d > n_bf
                    if variant == "norec" and d > 1:
                        chd = ch1
                    elif in_fp8:
                        chd = None  # fp8 slot written instead (below)
                    else:
                        chd = chpool.tile([P, HALF], bf16, tag="ch")
                    if d == 1:
                        nc.scalar.copy(chd[:], t[:])
                        ch1 = chd
                        cur = t
                    elif variant == "norec":
                        cur = None
                    else:
                        # pr = (T_{d-1} * 2) * t  (one fused DVE op)
                        pr = rpool.tile([P, HALF], f32, tag="rec")
                        nc.vector.scalar_tensor_tensor(
                            pr[:], tm1[:], 2.0, t[:], mult, mult
                        )
                        if d == 2:
                            # T_2 = pr - 1
                            cur = rpool.tile([P, HALF], f32, tag="rec")
                            nc.vector.tensor_scalar_sub(cur[:], pr[:], 1.0)
                            nc.scalar.copy(chd[:], cur[:])
                        elif in_fp8:
                            # fp8 degree: keep the fp32 value for the
                            # recursion, cast into this k-tile pair's
                            # DoubleRow slot (slot = i parity)
                            cur = rpool.tile([P, HALF], f32, tag="rec")
                            nc.vector.tensor_tensor(cur[:], pr[:], tm2[:], sub)
                            nc.scalar.copy(
                                ch8_list[i // 2][d][:, i % 2, :], cur[:]
                            )
                            if last:
                                cur = None
                        elif not last:
                            cur = rpool.tile([P, HALF], f32, tag="rec")
                            nc.vector.tensor_tensor(cur[:], pr[:], tm2[:], sub)
                            nc.scalar.copy(chd[:], cur[:])
                        else:
                            # final degree: write the bf16 tile directly
                            cur = None
                            nc.vector.tensor_tensor(chd[:], pr[:], tm2[:], sub)
                    tm2, tm1 = tm1, cur

                    if in_fp8:
                        continue  # consumed by the DoubleRow wave below
                    if variant == "nodma":
                        if i == 0 and d == 1:
                            wt0 = wpool.tile([P, 1, OUT_F], bf16, tag="w")
                            nc.sync.dma_start(wt0[:, 0], w[0, 0:P, :])
                        wt = wt0[:, 0]
                    else:
                        wt = wpool.tile([P, OUT_F], bf16, tag="w")
                        nc.sync.dma_start(wt[:], w[d - 1, i * P : (i + 1) * P, :])
                    if bm and last_i:
                        # bank-major tail: defer the last k-tile's matmuls so
                        # they can be issued per-bank (below), letting each
                        # bank's drain start while later banks still stream
                        chds[d] = chd
                        wts[d] = wt
                        continue
                    # under use_fp8 the accumulation group closes with the
                    # fp8 DoubleRow matmuls (emitted below), not here
                    stop = last_i and d == n_bf and not use_fp8
                    start = i == 0 and d == 1
                    for bc in range(NBC):
                        if variant == "oneldw":
                            # diagnostic: constant stationary everywhere so
                            # the post-compile dedup strips nearly every
                            # ldweights; numerics wrong, timing isolates the
                            # ldweights contribution
                            lhsT = ones[:]
                        else:
                            lhsT = chd[:, bc * P : (bc + 1) * P]
                        if wide:
                            nc.tensor.matmul(
                                ps[bc], lhsT, wt[:], start=start, stop=stop
                            )
                        else:
                            for oh in range(n_oh):
                                nc.tensor.matmul(
                                    ps[bc][oh],
                                    lhsT,
                                    wt[:, oh * 512 : (oh + 1) * 512],
                                    start=start,
                                    stop=stop,
                                )
                if use_fp8 and i % 2 == 1:
                    # This k-tile pair's fp8 degrees, interleaved into the
                    # same PSUM accumulation group: the bf16 weights are
                    # pre-scaled by 2^12 on the host (exact for bf16), so
                    # bf16 and fp8 contributions share one scale and one
                    # drain undoes it. No wave boundary, no extra drain.
                    j = i // 2
                    for dd_idx, d8 in enumerate(FP8_DEGS):
                        wt8 = w8pool.tile([P, 2, OUT_F], f8, tag="w8")
                        nc.sync.dma_start(wt8[:], w8[dd_idx, j])
                        stop8 = last_i and dd_idx == len(FP8_DEGS) - 1
                        for bc in range(NBC):
                            lhsT8 = ch8_list[j][d8][:, :, bc * P : (bc + 1) * P]
                            for oh in range(n_oh):
                                nc.tensor.matmul(
                                    ps[bc][oh],
                                    lhsT8,
                                    wt8[:, :, oh * 512 : (oh + 1) * 512],
                                    start=False,
                                    stop=stop8,
                                    perf_mode=mybir.MatmulPerfMode.DoubleRow,
                                )
                if bm and last_i:
                    # last k-tile, bank-major: bank bc finishes all its
                    # matmuls early and its drain (with the d=0 bias add)
                    # is emitted immediately, so the DVE drains overlap the
                    # remaining banks' matmul tail instead of serializing
                    # at the half boundary in front of the next half's
                    # recursion ops on the in-order DVE queue.
                    for bc in range(NBC):
                        for d in range(1, DEG + 1):
                            lhsT = chds[d][:, bc * P : (bc + 1) * P]
                            for oh in range(n_oh):
                                nc.tensor.matmul(
                                    ps[bc][oh],
                                    lhsT,
                                    wts[d][:, oh * 512 : (oh + 1) * 512],
                                    start=False,
                                    stop=d == DEG,
                                )
                        if variant == "nodrain":
                            continue
                        for oh in range(n_oh):
                            ot = opool.tile([P, 512], f32, tag="ot")
                            nc.vector.tensor_tensor(
                                ot[:],
                                ps[bc][oh],
                                biasrep[:, oh * 512 : (oh + 1) * 512],
                                add,
                            )
                            r0 = h * HALF + bc * P
                            nc.sync.dma_start(
                                out[r0 : r0 + P, oh * 512 : (oh + 1) * 512],
                                ot[:],
                            )
            if bm:
                continue  # drains already emitted per-bank above
            if variant == "nodrain":
                continue
            if use_fp8:
                # single accumulation group at 2^12 scale (bf16 weights are
                # host-pre-scaled, exactly): one drain undoes the scale and
                # adds the d=0 bias
                for bc in range(NBC):
                    for oh in range(n_oh):
                        ot = opool.tile([P, 512], f32, tag="ot")
                        nc.vector.scalar_tensor_tensor(
                            ot[:], ps[bc][oh], 2.0 ** -12,
                            biasrep[:, oh * 512 : (oh + 1) * 512], mult, add,
                        )
                        r0 = h * HALF + bc * P
                        nc.sync.dma_start(
                            out[r0 : r0 + P, oh * 512 : (oh + 1) * 512],
                            ot[:],
                        )
                continue
            # Drain this half's PSUM to SBUF (adding the d=0 bias) and then
            # HBM. The bias-add rides the drain copy for free on DVE.
            for bc in range(NBC):
                for oh in range(n_oh):
                    ot = opool.tile([P, 512], f32, tag="ot")
                    src = (
                        ps[bc][:, oh * 512 : (oh + 1) * 512]
                        if wide
                        else ps[bc][oh]
                    )
                    nc.vector.tensor_tensor(
                        ot[:], src, biasrep[:, oh * 512 : (oh + 1) * 512], add
                    )
                    r0 = h * HALF + bc * P
                    nc.sync.dma_start(
                        out[r0 : r0 + P, oh * 512 : (oh + 1) * 512], ot[:]
                    )


def _get_nc(loop_r=None, variant=""):
    key = (loop_r, variant)
    if key not in _CACHED_NC:
        _CACHED_NC[key] = _build_bass(loop_r, variant)
    return _CACHED_NC[key]


def _prep_inputs(x, coefficients):
    bf16 = ml_dtypes.bfloat16
    e4 = ml_dtypes.float8_e4m3
    SC8 = 2.0 ** 12
    x = np.asarray(x, dtype=np.float32)
    coef = np.asarray(coefficients, dtype=np.float32)
    # (d, i, o) bf16 for d = 1..DEG, pre-scaled by 2^12 (exact for bf16: a
    # power-of-2 scale leaves the mantissa untouched) so the bf16 and fp8
    # contributions accumulate at one scale; the drain multiplies by 2^-12.
    # Degrees 7..8 are unused by the default variant but kept so diagnostic
    # variants stay runnable.
    w_all = np.ascontiguousarray(
        coef.transpose(2, 0, 1)[1 : DEG + 1] * np.float32(SC8)
    ).astype(bf16)
    # degrees 7..8 in fp8 e4m3 (DoubleRow): k-tile pairs j=(2j, 2j+1) on a
    # separate axis, coefficients pre-scaled by 2^12 so they use e4m3's
    # normal range (sigma*2^12 ~ 0.44); the drain merge multiplies by 2^-12
    w8_arr = np.zeros((len(FP8_DEGS), NI // 2, P, 2, OUT_F), dtype=e4)
    for dd, d in enumerate(FP8_DEGS):
        for j in range(NI // 2):
            for s in range(2):
                blk = coef[(2 * j + s) * P : (2 * j + s + 1) * P, :, d] * SC8
                w8_arr[dd, j, :, s, :] = blk.astype(e4)
    # d=0 term is a per-output bias (T_0 == 1): summed over i on the host,
    # replicated across the 128 partitions, added during the PSUM drain
    bias = coef[:, :, 0].sum(axis=0, dtype=np.float64).astype(np.float32)
    brep_arr = np.ascontiguousarray(
        np.broadcast_to(bias, (P, OUT_F)), dtype=np.float32
    )
    in_maps = []
    for c in range(N_CORES):
        xc = x[c * B_CORE : (c + 1) * B_CORE, :]
        in_maps.append(
            {
                "xt": np.ascontiguousarray(xc.T),
                "w": w_all,
                "w8": w8_arr,
                "brep": brep_arr,
            }
        )
    return in_maps


VARIANT = ""  # production variant used by kernel()/run()


def run(x, coefficients, trace=False, tmpdir=None):
    """Run on hardware; returns (out, BassKernelResults)."""
    from concourse.bass_utils import run_bass_kernel_spmd

    nc = _get_nc(None, VARIANT)
    in_maps = _prep_inputs(x, coefficients)
    res = run_bass_kernel_spmd(
        nc,
        in_maps,
        core_ids=list(range(N_CORES)),
        trace=trace,
        tmpdir=tmpdir,
    )
    out = np.concatenate([r["out"] for r in res.results], axis=0)
    return np.ascontiguousarray(out, dtype=np.float32), res


def kernel(x, coefficients):
    out, _ = run(x, coefficients, trace=False)
    return out



# revision 34
# speedup vs baseline: 1.0349x; 1.0349x over previous
"""ChebyKAN linear layer on 8 Trainium2 NeuronCores.

Computation: out[b,o] = sum_{i,d} T_d(tanh(x[b,i])) * coef[i,o,d]
  == sum_d T_d(tanh(x)) @ C_d   (9 accumulated 8192x1024x1024 matmuls)

Strategy:
  - Data-parallel over batch: core c handles rows [c*1024, (c+1)*1024).
  - Host pre-transposes each core's x slice to (in_features, batch) layout so
    the contraction dim (i) lands on SBUF partitions, and repacks the
    coefficients to (d, i, o): bf16 for degrees 1..5, fp8 e4m3 (k-tile
    pairs on a separate axis for DoubleRow) for degrees 6..8. ALL weights
    are pre-scaled by 2^12 — exact for bf16 (power-of-2 scale leaves the
    mantissa untouched) and required for fp8 (coef sigma ~1.1e-4 would land
    in e4m3's subnormal/flush range unscaled) — so every matmul accumulates
    into one PSUM group at a single scale.
  - On-chip: ACT computes tanh in fp32, DVE runs the Chebyshev recursion
    T_d = 2 t T_{d-1} - T_{d-2} in fp32 (scalar_tensor_tensor fuses the
    2*t*T_{d-1} product into one op), ACT casts T_1..T_5 to bf16 and
    T_6..T_8 into slot i%2 of fp8 e4m3 DoubleRow pair tiles [128, 2, 512].
  - PE: per half, one PSUM accumulation group per (b-chunk, o-half) bank:
    bf16 matmuls for degrees 1..5 stream per k-tile; after each odd k-tile
    the pair's fp8 DoubleRow matmuls (2 k-tiles per instruction, 2x MACs
    per cycle-row) are interleaved into the same group. One DVE drain per
    bank computes psum * 2^-12 + bias (d=0 term, host-precomputed fp32,
    partition-replicated) and DMAs out.
  - Per core the 1024-row batch is processed in two 512-column halves; each
    half keeps its full output (4 b-chunks x 2 o-halves) resident in all
    8 PSUM banks while the k-blocks accumulate into it.

Numerics (validated on HW, matches a numpy simulation of the exact
quantization to ~1e-6): rel l2 error vs fp32 reference 1.80e-2 (threshold
2e-2). fp8 on 3 of 8 degrees contributes ~sqrt(3)*1.04e-2, the bf16
remainder ~1.8e-3; 4 fp8 degrees would be 2.07e-2 — over threshold.

Performance notes (8-core SPMD, measured via on-device For_i loop slope
because the axon tunnel's ~80-95 ms RPC overhead hides the kernel and NTFF
profiling is unavailable through it; cross-process slope noise is +-5 us):
  - Measured matmult throughput on this part (mmonly/mmrows256 diagnostic
    variants, pure PE stream, no deps) is 0.51 ns/row — 1.96 GHz effective,
    NOT the 2.4 GHz nominal — and is linear in rows with ~zero per-
    instruction overhead. The all-bf16 kernel (1024 512-row matmuls,
    267 us of stream) measured 269-283 us: already ~97% of the PE roofline.
  - Ldweights are nearly free when sync-free (hidden in the matmul
    pipeline): deduping 187 extra reloads moved the time < 2 us. The
    engine-aware dedup pass below keeps them at the 1-per-(bc-chunk) floor.
  - DMA (weights re-streamed per half, 33.6 MB/iter/core), PSUM drains and
    half boundaries are all second-order (nodma/nodrain variants within
    noise of base).
  - fp8 DoubleRow halves the row count of each converted degree. Measured:
    all-bf16 ~274 us; degrees {7,8} fp8 (two-wave drain) 248.8 us; degrees
    {6,7,8} fp8 (two-wave) 234.4 us; single-group (current) removes the
    wave-split drains and bubbles.
  - A 1024-free matmult (2 PSUM banks) is ISA-illegal (s3d3_mm_num_elements
    codegen assert); pure-fp8 everything fails accuracy (3.1e-2).
"""

import numpy as np
import ml_dtypes

BATCH = 8192
IN_F = 1024
OUT_F = 1024
DEG = 8  # degree; DEG+1 coefficients per (i,o)
FP8_DEGS = (6, 7, 8)  # degrees computed as fp8 e4m3 DoubleRow
N_CORES = 8
B_CORE = BATCH // N_CORES  # 1024
P = 128
HALF = 512  # batch columns processed per PSUM-resident output block
NI = IN_F // P  # 8 contraction tiles
NBC = HALF // P  # 4 b-chunks per half
NOH = OUT_F // 512  # 2 output halves of 512
N_HALF = B_CORE // HALF  # 2

_CACHED_NC = {}


def _build_bass(loop_r=None, variant=""):
    """Build the Bass program. loop_r wraps the whole compute in a hardware
    For loop of loop_r iterations (benchmark-only; slope over loop_r gives
    per-iteration HW time since the axon RPC overhead is per-call)."""
    import contextlib

    import concourse.mybir as mybir
    import concourse.tile as tile
    from concourse import bacc

    f32 = mybir.dt.float32
    bf16 = mybir.dt.bfloat16
    mult = mybir.AluOpType.mult
    sub = mybir.AluOpType.subtract
    Tanh = mybir.ActivationFunctionType.Tanh

    import json as _json

    def _dedup_ldweights(b):
        """Remove InstLdweights that reload the identical stationary operand
        still held by the PE array. Tile emits one Ldweights per matmul, so a
        weight reused by consecutive matmuls is loaded twice; each redundant
        load costs ~55 ns of serial PE time. Only sync-free exact duplicates
        are removed. Instructions on OTHER engine queues interleaved in the
        block's program order cannot disturb the PE weight registers, so they
        do not reset the tracked key; the only PE-queue instructions between
        a duplicate pair are Matmult (does not disturb weights) and sync-free
        EventSemaphore. An EventSemaphore with a wait is treated as a fence
        (conservative: it could order an SBUF rewrite of the stationary)."""
        n_removed = 0
        PE = mybir.EngineType.PE
        for fn in b.m.functions:
            for blk in fn.blocks:
                last_key = None
                keep = []
                for inst in blk.instructions:
                    if isinstance(inst, mybir.InstLdweights):
                        d = _json.loads(
                            mybir.instruction_to_pretty_json_string(inst)
                        )
                        si = d.get("sync_info") or {}
                        has_sync = bool(
                            si.get("on_wait") or si.get("on_update")
                        )
                        key = _json.dumps(
                            [
                                d.get("ins"),
                                d.get("perf_mode"),
                                d.get("is_transpose"),
                                d.get("tile_position"),
                                d.get("tile_size"),
                            ],
                            sort_keys=True,
                        )
                        if key == last_key and not has_sync:
                            n_removed += 1
                            continue
                        last_key = key
                    elif inst.engine != PE:
                        pass  # other-engine work; PE array unaffected
                    elif isinstance(inst, mybir.InstMatmult):
                        pass  # matmult does not disturb loaded weights
                    elif isinstance(inst, mybir.InstEventSemaphore):
                        si = inst.sync_info
                        if si is not None and si.on_wait:
                            # conservative fence: a PE-queue wait could
                            # order an SBUF rewrite of the stationary
                            last_key = None
                    else:
                        last_key = None
                    keep.append(inst)
                blk.instructions[:] = keep
        b._ldw_removed = n_removed

    class _Bacc(bacc.Bacc):
        def compile(self):
            super().compile()
            _dedup_ldweights(self)

    f8 = mybir.dt.float8e4

    nc = _Bacc(name="chebykan")
    xt = nc.dram_tensor("xt", (IN_F, B_CORE), f32, kind="ExternalInput")
    w = nc.dram_tensor("w", (DEG, IN_F, OUT_F), bf16, kind="ExternalInput")
    # FP8_DEGS as fp8 e4m3, k-tile pairs on a separate axis for DoubleRow:
    # w8[dd, j, k, s, o] = coef[(2j+s)*128 + k, o, FP8_DEGS[dd]] * 2^12
    w8 = nc.dram_tensor(
        "w8", (len(FP8_DEGS), NI // 2, P, 2, OUT_F), f8, kind="ExternalInput"
    )
    brep = nc.dram_tensor("brep", (P, OUT_F), f32, kind="ExternalInput")
    out = nc.dram_tensor("out", (B_CORE, OUT_F), f32, kind="ExternalOutput")

    with (
        tile.TileContext(nc) as tc,
        tc.tile_pool(name="wpool", bufs=12) as wpool,
        tc.tile_pool(name="w8pool", bufs=6) as w8pool,
        tc.tile_pool(name="xpool", bufs=8) as xpool,
        tc.tile_pool(name="tanh", bufs=3) as tanpool,
        tc.tile_pool(name="rec", bufs=6) as rpool,
        tc.tile_pool(name="ch", bufs=16) as chpool,
        tc.tile_pool(name="ch8", bufs=16) as ch8pool,
        tc.tile_pool(name="const", bufs=1) as cpool,
        tc.tile_pool(name="outp", bufs=16) as opool,
        tc.tile_pool(name="psum", bufs=1, space="PSUM") as pspool,
    ):
        ones = cpool.tile([P, P], bf16)
        nc.vector.memset(ones[:], 1.0)
        # d=0 bias, pre-replicated across partitions on the host; added
        # during the PSUM drain instead of spending PE matmuls on it
        biasrep = cpool.tile([P, OUT_F], f32)
        nc.sync.dma_start(biasrep[:], brep[:, :])

        loop_cm = (
            tc.For_i(
                0,
                loop_r,
                1,
                hint_engines=(mybir.EngineType.PE, mybir.EngineType.SP),
            )
            if loop_r is not None
            else contextlib.nullcontext()
        )
        with loop_cm:
            _emit_body(nc, tc, xt, w, w8, out, ones, biasrep,
                       wpool, w8pool, xpool, tanpool, rpool, chpool, ch8pool,
                       opool, pspool, f32, bf16, f8, mult, sub, Tanh, variant)
    nc.finalize()
    return nc


def _emit_body(nc, tc, xt, w, w8, out, ones, biasrep,
               wpool, w8pool, xpool, tanpool, rpool, chpool, ch8pool,
               opool, pspool, f32, bf16, f8, mult, sub, Tanh, variant=""):
    import concourse.mybir as mybir
    add = mybir.AluOpType.add
    wide = variant == "wide"
    n_oh = 1 if variant == "halfmm" else NOH
    if variant in ("mmonly", "mmrows256"):
        # diagnostic: pure PE stream — same matmult/psum-group structure as
        # the real kernel but constant operands, no DMA/recursion/drains.
        # Measures sustained matmult throughput on this hardware.
        nfree = 256 if variant == "mmrows256" else 512
        wt0 = wpool.tile([P, OUT_F], bf16, tag="w")
        nc.sync.dma_start(wt0[:], w[0, 0:P, :])
        for h in range(N_HALF):
            ps = [
                [
                    pspool.tile([P, nfree], f32, tag=f"psm_{bc}_{oh}",
                                name=f"psm_{bc}_{oh}")
                    for oh in range(NOH)
                ]
                for bc in range(NBC)
            ]
            for i in range(NI):
                for d in range(1, DEG + 1):
                    start = i == 0 and d == 1
                    stop = i == NI - 1 and d == DEG
                    for bc in range(NBC):
                        for oh in range(NOH):
                            nc.tensor.matmul(
                                ps[bc][oh],
                                ones,
                                wt0[:, oh * nfree : (oh + 1) * nfree],
                                start=start,
                                stop=stop,
                            )
        return
    for h in range(N_HALF):
            if wide:
                # one [P, 1024] tile spans 2 PSUM banks; a single matmult
                # accumulates the full 1024-wide output row block
                ps = [
                    pspool.tile([P, OUT_F], f32, tag=f"psw_{bc}",
                                name=f"psw_{bc}")
                    for bc in range(NBC)
                ]
            else:
                ps = [
                    [
                        pspool.tile(
                            [P, 512], f32, tag=f"ps_{bc}_{oh}",
                            name=f"ps_{bc}_{oh}"
                        )
                        for oh in range(n_oh)
                    ]
                    for bc in range(NBC)
                ]
            bm = variant == "bm"
            use_fp8 = variant in ("", "fp8")
            n_bf = DEG - len(FP8_DEGS) if use_fp8 else DEG  # degrees in bf16
            ch8_list = []  # per k-tile pair j: {7: tile, 8: tile}
            for i in range(NI):
                last_i = i == NI - 1
                chds = {}
                wts = {}
                if use_fp8 and i % 2 == 0:
                    pair = {}
                    for dd in FP8_DEGS:
                        pair[dd] = ch8pool.tile(
                            [P, 2, HALF], f8, tag=f"ch8_{dd}",
                            name=f"ch8_{h}_{i}_{dd}",
                        )
                    ch8_list.append(pair)
                xti = xpool.tile([P, HALF], f32, tag="x")
                nc.sync.dma_start(
                    xti[:], xt[i * P : (i + 1) * P, h * HALF : (h + 1) * HALF]
                )
                t = tanpool.tile([P, HALF], f32, tag="t")
                nc.scalar.activation(t[:], xti[:], Tanh)

                tm2 = None  # T_{d-2} (fp32); None encodes T_0 == 1
                tm1 = t  # T_{d-1} (fp32)
                ch1 = None
                for d in range(1, DEG + 1):
                    last = d == DEG
                    in_fp8 = use_fp8 and d > n_bf
                    if variant == "norec" and d > 1:
                        chd = ch1
                    elif in_fp8:
                        chd = None  # fp8 slot written instead (below)
                    else:
                        chd = chpool.tile([P, HALF], bf16, tag="ch")
                    if d == 1:
                        nc.scalar.copy(chd[:], t[:])
                        ch1 = chd
                        cur = t
                    elif variant == "norec":
                        cur = None
                    else:
                        # pr = (T_{d-1} * 2) * t  (one fused DVE op)
                        pr = rpool.tile([P, HALF], f32, tag="rec")
                        nc.vector.scalar_tensor_tensor(
                            pr[:], tm1[:], 2.0, t[:], mult, mult
                        )
                        if d == 2:
                            # T_2 = pr - 1
                            cur = rpool.tile([P, HALF], f32, tag="rec")
                            nc.vector.tensor_scalar_sub(cur[:], pr[:], 1.0)
                            nc.scalar.copy(chd[:], cur[:])
                        elif in_fp8:
                            # fp8 degree: keep the fp32 value for the
                            # recursion, cast into this k-tile pair's
                            # DoubleRow slot (slot = i parity)
                            cur = rpool.tile([P, HALF], f32, tag="rec")
                            nc.vector.tensor_tensor(cur[:], pr[:], tm2[:], sub)
                            nc.scalar.copy(
                                ch8_list[i // 2][d][:, i % 2, :], cur[:]
                            )
                            if last:
                                cur = None
                        elif not last:
                            cur = rpool.tile([P, HALF], f32, tag="rec")
                            nc.vector.tensor_tensor(cur[:], pr[:], tm2[:], sub)
                            nc.scalar.copy(chd[:], cur[:])
                        else:
                            # final degree: write the bf16 tile directly
                            cur = None
                            nc.vector.tensor_tensor(chd[:], pr[:], tm2[:], sub)
                    tm2, tm1 = tm1, cur

                    if in_fp8:
                        continue  # consumed by the DoubleRow wave below
                    if variant == "nodma":
                        if i == 0 and d == 1:
                            wt0 = wpool.tile([P, 1, OUT_F], bf16, tag="w")
                            nc.sync.dma_start(wt0[:, 0], w[0, 0:P, :])
                        wt = wt0[:, 0]
                    else:
                        wt = wpool.tile([P, OUT_F], bf16, tag="w")
                        nc.sync.dma_start(wt[:], w[d - 1, i * P : (i + 1) * P, :])
                    if bm and last_i:
                        # bank-major tail: defer the last k-tile's matmuls so
                        # they can be issued per-bank (below), letting each
                        # bank's drain start while later banks still stream
                        chds[d] = chd
                        wts[d] = wt
                        continue
                    # under use_fp8 the accumulation group closes with the
                    # fp8 DoubleRow matmuls (emitted below), not here
                    stop = last_i and d == n_bf and not use_fp8
                    start = i == 0 and d == 1
                    for bc in range(NBC):
                        if variant == "oneldw":
                            # diagnostic: constant stationary everywhere so
                            # the post-compile dedup strips nearly every
                            # ldweights; numerics wrong, timing isolates the
                            # ldweights contribution
                            lhsT = ones[:]
                        else:
                            lhsT = chd[:, bc * P : (bc + 1) * P]
                        if wide:
                            nc.tensor.matmul(
                                ps[bc], lhsT, wt[:], start=start, stop=stop
                            )
                        else:
                            for oh in range(n_oh):
                                nc.tensor.matmul(
                                    ps[bc][oh],
                                    lhsT,
                                    wt[:, oh * 512 : (oh + 1) * 512],
                                    start=start,
                                    stop=stop,
                                )
                if use_fp8 and i % 2 == 1:
                    # This k-tile pair's fp8 degrees, interleaved into the
                    # same PSUM accumulation group: the bf16 weights are
                    # pre-scaled by 2^12 on the host (exact for bf16), so
                    # bf16 and fp8 contributions share one scale and one
                    # drain undoes it. No wave boundary, no extra drain.
                    j = i // 2
                    for dd_idx, d8 in enumerate(FP8_DEGS):
                        wt8 = w8pool.tile([P, 2, OUT_F], f8, tag="w8")
                        nc.sync.dma_start(wt8[:], w8[dd_idx, j])
                        stop8 = last_i and dd_idx == len(FP8_DEGS) - 1
                        for bc in range(NBC):
                            lhsT8 = ch8_list[j][d8][:, :, bc * P : (bc + 1) * P]
                            for oh in range(n_oh):
                                nc.tensor.matmul(
                                    ps[bc][oh],
                                    lhsT8,
                                    wt8[:, :, oh * 512 : (oh + 1) * 512],
                                    start=False,
                                    stop=stop8,
                                    perf_mode=mybir.MatmulPerfMode.DoubleRow,
                                )
                if bm and last_i:
                    # last k-tile, bank-major: bank bc finishes all its
                    # matmuls early and its drain (with the d=0 bias add)
                    # is emitted immediately, so the DVE drains overlap the
                    # remaining banks' matmul tail instead of serializing
                    # at the half boundary in front of the next half's
                    # recursion ops on the in-order DVE queue.
                    for bc in range(NBC):
                        for d in range(1, DEG + 1):
                            lhsT = chds[d][:, bc * P : (bc + 1) * P]
                            for oh in range(n_oh):
                                nc.tensor.matmul(
                                    ps[bc][oh],
                                    lhsT,
                                    wts[d][:, oh * 512 : (oh + 1) * 512],
                                    start=False,
                                    stop=d == DEG,
                                )
                        if variant == "nodrain":
                            continue
                        for oh in range(n_oh):
                            ot = opool.tile([P, 512], f32, tag="ot")
                            nc.vector.tensor_tensor(
                                ot[:],
                                ps[bc][oh],
                                biasrep[:, oh * 512 : (oh + 1) * 512],
                                add,
                            )
                            r0 = h * HALF + bc * P
                            nc.sync.dma_start(
                                out[r0 : r0 + P, oh * 512 : (oh + 1) * 512],
                                ot[:],
                            )
            if bm:
                continue  # drains already emitted per-bank above
            if variant == "nodrain":
                continue
            if use_fp8:
                # single accumulation group at 2^12 scale (bf16 weights are
                # host-pre-scaled, exactly): one drain undoes the scale and
                # adds the d=0 bias
                for bc in range(NBC):
                    for oh in range(n_oh):
                        ot = opool.tile([P, 512], f32, tag="ot")
                        nc.vector.scalar_tensor_tensor(
                            ot[:], ps[bc][oh], 2.0 ** -12,
                            biasrep[:, oh * 512 : (oh + 1) * 512], mult, add,
                        )
                        r0 = h * HALF + bc * P
                        nc.sync.dma_start(
                            out[r0 : r0 + P, oh * 512 : (oh + 1) * 512],
                            ot[:],
                        )
                continue
            # Drain this half's PSUM to SBUF (adding the d=0 bias) and then
            # HBM. The bias-add rides the drain copy for free on DVE.
            for bc in range(NBC):
                for oh in range(n_oh):
                    ot = opool.tile([P, 512], f32, tag="ot")
                    src = (
                        ps[bc][:, oh * 512 : (oh + 1) * 512]
                        if wide
                        else ps[bc][oh]
                    )
                    nc.vector.tensor_tensor(
                        ot[:], src, biasrep[:, oh * 512 : (oh + 1) * 512], add
                    )
                    r0 = h * HALF + bc * P
                    nc.sync.dma_start(
                        out[r0 : r0 + P, oh * 512 : (oh + 1) * 512], ot[:]
                    )


def _get_nc(loop_r=None, variant=""):
    key = (loop_r, variant)
    if key not in _CACHED_NC:
        _CACHED_NC[key] = _build_bass(loop_r, variant)
    return _CACHED_NC[key]


def _prep_inputs(x, coefficients):
    bf16 = ml_dtypes.bfloat16
    e4 = ml_dtypes.float8_e4m3
    SC8 = 2.0 ** 12
    x = np.asarray(x, dtype=np.float32)
    coef = np.asarray(coefficients, dtype=np.float32)
    # (d, i, o) bf16 for d = 1..DEG, pre-scaled by 2^12 (exact for bf16: a
    # power-of-2 scale leaves the mantissa untouched) so the bf16 and fp8
    # contributions accumulate at one scale; the drain multiplies by 2^-12.
    # Degrees 7..8 are unused by the default variant but kept so diagnostic
    # variants stay runnable.
    w_all = np.ascontiguousarray(
        coef.transpose(2, 0, 1)[1 : DEG + 1] * np.float32(SC8)
    ).astype(bf16)
    # degrees 7..8 in fp8 e4m3 (DoubleRow): k-tile pairs j=(2j, 2j+1) on a
    # separate axis, coefficients pre-scaled by 2^12 so they use e4m3's
    # normal range (sigma*2^12 ~ 0.44); the drain merge multiplies by 2^-12
    w8_arr = np.zeros((len(FP8_DEGS), NI // 2, P, 2, OUT_F), dtype=e4)
    for dd, d in enumerate(FP8_DEGS):
        for j in range(NI // 2):
            for s in range(2):
                blk = coef[(2 * j + s) * P : (2 * j + s + 1) * P, :, d] * SC8
                w8_arr[dd, j, :, s, :] = blk.astype(e4)
    # d=0 term is a per-output bias (T_0 == 1): summed over i on the host,
    # replicated across the 128 partitions, added during the PSUM drain
    bias = coef[:, :, 0].sum(axis=0, dtype=np.float64).astype(np.float32)
    brep_arr = np.ascontiguousarray(
        np.broadcast_to(bias, (P, OUT_F)), dtype=np.float32
    )
    in_maps = []
    for c in range(N_CORES):
        xc = x[c * B_CORE : (c + 1) * B_CORE, :]
        in_maps.append(
            {
                "xt": np.ascontiguousarray(xc.T),
                "w": w_all,
                "w8": w8_arr,
                "brep": brep_arr,
            }
        )
    return in_maps


VARIANT = ""  # production variant used by kernel()/run()


def run(x, coefficients, trace=False, tmpdir=None):
    """Run on hardware; returns (out, BassKernelResults)."""
    from concourse.bass_utils import run_bass_kernel_spmd

    nc = _get_nc(None, VARIANT)
    in_maps = _prep_inputs(x, coefficients)
    res = run_bass_kernel_spmd(
        nc,
        in_maps,
        core_ids=list(range(N_CORES)),
        trace=trace,
        tmpdir=tmpdir,
    )
    out = np.concatenate([r["out"] for r in res.results], axis=0)
    return np.ascontiguousarray(out, dtype=np.float32), res


def kernel(x, coefficients):
    out, _ = run(x, coefficients, trace=False)
    return out

